# revision 61
# baseline (speedup 1.0000x reference)
"""Distributed (8-core) Trainium2 Bass kernel for nn_Attention.

Reference computation (per batch b of 4, x: [4, 256, 2048]):
  qkv = w_qkv @ x[b]            -> q,k,v each [8 heads, 64, 2048]
  dots = (q^T k) * 64**-0.5     -> [8, 2048, 2048]
  attn = softmax(dots, -1)
  av   = v @ attn^T             -> [8, 64, 2048]
  out  = w_out @ av + b_out     -> [256, 2048]

Sharding: 8 shards = (batch b in 0..3) x (query-half in 0..1). Each core
gets the full x[b] (columns permuted so its own 1024 query positions come
first), computes full k/v (duplicated with its half-partner), q only for
its 1024 queries, its half of the attention, and its half of the final
projection. Host concatenates.

The kernel is ACT-bound: 128 exp ACTIVATEs of [128,1024] at ~1.11us each
(softmax exp over 16.8M scores/core) = 142us of ScalarE work. Everything
else is scheduled to keep that stream gapless:
  - flat 128-step stream over (pair, ic, jc); per step s the emission
    order is exp(s), dots(s+1), extras, AV(s) so the next exp's input
    (dots) never queues behind the e(s)-dependent AV matmuls in the
    in-order PE queue.
  - q/k projections, v^T projections, out-projections and normalize
    broadcasts are interleaved 1-2 matmuls per step into PE slack
    instead of bursts.
  - softmax denominators ride as a ones-column in the AV stationary
    operand; normalization is copy-out (frees the PSUM bank), recip
    (DVE), bf16 cast, ones-matmul partition broadcast (PE), multiply
    (DVE). No GPSIMD on the critical path.
"""

import sys

sys.path.insert(0, "/opt/trn_rl_repo")
sys.path.insert(0, "/root/.axon_site")

import numpy as np

DIM = 256
N = 2048
NQ = 1024
H = 8
DH = 64
HID = 512
PAIRS = 4
CHAINS = 8  # (pair, ic)
STEPS = CHAINS * 16
SCALE = DH ** -0.5

_CACHE = {}


def _register_ntff_hook():
    """The agent image's antenv lacks axon_hooks; synthesize it so
    run_bass_kernel_spmd(trace=True) can profile. Harmless if unused."""
    import types

    if "antenv.axon_hooks" in sys.modules:
        return
    try:
        import antenv
        from trn_agent_boot.trn_boot import _ntff_profile_via_ctypes

        mod = types.ModuleType("antenv.axon_hooks")
        _hook = [None]
        mod.set_axon_ntff_profile_hook = lambda h: _hook.__setitem__(0, h)
        mod.get_axon_ntff_profile_hook = lambda: _hook[0]
        sys.modules["antenv.axon_hooks"] = mod
        antenv.axon_hooks = mod
        mod.set_axon_ntff_profile_hook(
            _ntff_profile_via_ctypes("/opt/axon/libaxon_pjrt.so")
        )
    except Exception:
        pass


def build_nc():
    import concourse.mybir as mybir
    import concourse.tile as tile
    from concourse import bacc

    f32 = mybir.dt.float32
    bf16 = mybir.dt.bfloat16
    Exp = mybir.ActivationFunctionType.Exp

    nc = bacc.Bacc("TRN2", target_bir_lowering=False, debug=False)

    x_ext = nc.dram_tensor("x", [DIM, N], bf16, kind="ExternalInput")
    wq_ext = nc.dram_tensor("wq_t", [DIM, HID], bf16, kind="ExternalInput")
    wk_ext = nc.dram_tensor("wk_t", [DIM, HID], bf16, kind="ExternalInput")
    wv_ext = nc.dram_tensor("wv_t", [DIM, HID], bf16, kind="ExternalInput")
    wo_ext = nc.dram_tensor("wo_t", [HID, DIM], bf16, kind="ExternalInput")
    b_ext = nc.dram_tensor("bias", [DIM, 1], f32, kind="ExternalInput")
    out_ext = nc.dram_tensor("out", [DIM, NQ], f32, kind="ExternalOutput")

    VSLOT = DH + 1  # 64 v columns + 1 ones column per head

    with tile.TileContext(nc) as tc:
        with (
            tc.tile_pool(name="persist", bufs=1) as pp,
            tc.tile_pool(name="qk", bufs=2) as qk,
            tc.tile_pool(name="epool", bufs=20) as ep,
            tc.tile_pool(name="small", bufs=4) as sp,
            tc.tile_pool(name="pdots", bufs=2, space="PSUM") as pd,
            tc.tile_pool(name="pattn", bufs=2, space="PSUM") as pa,
            tc.tile_pool(name="pproj", bufs=1, space="PSUM") as pj,
            tc.tile_pool(name="pvt", bufs=1, space="PSUM") as pv,
        ):
            # ---- warm the ACT exp table early (one tiny op) ----
            dummy = sp.tile([1, 1], f32, tag="dummy")
            nc.vector.memset(dummy[:], 0.0)
            dummy2 = sp.tile([1, 1], f32, tag="dummy2")
            nc.scalar.activation(dummy2[:], dummy[:], Exp)

            # ---- input DMAs: wq + x head first so the q projection can
            # start ASAP; wk/wv next (k proj, v proj); bulk x after.
            engs = [nc.sync, nc.gpsimd]
            wq_sb = [pp.tile([128, HID], bf16, tag=f"wq{c}", name=f"wq{c}") for c in range(2)]
            wk_sb = [pp.tile([128, HID], bf16, tag=f"wk{c}", name=f"wk{c}") for c in range(2)]
            wv_sb = [pp.tile([128, HID], bf16, tag=f"wv{c}", name=f"wv{c}") for c in range(2)]
            # x split into 3 column-group tiles per row chunk: dependency
            # tracking is tile-granular, so a single [128,2048] tile would
            # make every consumer wait for ALL of x's DMAs.
            XW = [512, 512, 1024]
            XO = [0, 512, 1024]
            xg = [
                [
                    pp.tile([128, XW[g]], bf16, tag=f"x{c}_{g}", name=f"x{c}_{g}")
                    for g in range(3)
                ]
                for c in range(2)
            ]

            def x_rhs(cc, col, width):
                g = 0 if col < 512 else (1 if col < 1024 else 2)
                off = col - XO[g]
                return xg[cc][g][:, off : off + width]

            # 3 DGE queues; the q/k projection inputs (wq, xA, wk) are the
            # startup critical path, one ~256KB set per queue.
            for c in range(2):
                engs[c].dma_start(wq_sb[c][:], wq_ext[c * 128 : (c + 1) * 128, :])
            for c in range(2):
                engs[c].dma_start(
                    xg[c][0][:], x_ext[c * 128 : (c + 1) * 128, 0:512]
                )
            for c in range(2):
                engs[c].dma_start(wk_sb[c][:], wk_ext[c * 128 : (c + 1) * 128, :])
            for c in range(2):
                engs[c].dma_start(wv_sb[c][:], wv_ext[c * 128 : (c + 1) * 128, :])
            for c in range(2):
                engs[c].dma_start(
                    xg[c][1][:], x_ext[c * 128 : (c + 1) * 128, 512:1024]
                )
            for c in range(2):
                engs[c].dma_start(
                    xg[c][2][:], x_ext[c * 128 : (c + 1) * 128, 1024:2048]
                )
            wo_sb = []
            for cc in range(4):
                t = pp.tile([128, DIM], bf16, tag=f"wo{cc}", name=f"wo{cc}")
                engs[cc % 2].dma_start(t[:], wo_ext[cc * 128 : (cc + 1) * 128, :])
                wo_sb.append(t)
            bias_sb = pp.tile([128, 2], f32, tag="bias")
            for oc in range(2):
                nc.sync.dma_start(
                    bias_sb[:, oc : oc + 1], b_ext[oc * 128 : (oc + 1) * 128, :]
                )

            # ---- persistent SBUF state ----
            ones_col = pp.tile([1, DH], f32, tag="ones_col")
            nc.vector.memset(ones_col[:], 1.0)
            ones_sb = pp.tile([128, H], f32, tag="ones")
            nc.vector.memset(ones_sb[:], 1.0)
            vt = pp.tile([128, 16 * H * VSLOT], bf16, tag="vt")

            attn_n = [
                pp.tile([128, NQ], bf16, tag=f"attn_n{p}", name=f"attn_n{p}")
                for p in range(PAIRS)
            ]
            out_acc = [
                pp.tile([128, NQ], f32, tag=f"oacc{oc}", name=f"oacc{oc}")
                for oc in range(2)
            ]

            # chain bookkeeping: chain c = (pair c//2, ic c%2)
            qk_tiles = [None] * PAIRS
            att_tiles = [None] * CHAINS
            e_tiles = [None] * STEPS
            norm_state = {}

            # ---- helper unit emitters ----
            def qk_unit(p, which, col, pool, width=512):
                """One q/k projection unit for pair p: [128,width] + cast."""
                if qk_tiles[p] is None:
                    qk_tiles[p] = (
                        qk.tile([128, NQ], bf16, tag="q", name=f"q{p}"),
                        qk.tile([128, N], bf16, tag="k", name=f"k{p}"),
                    )
                dst_t = qk_tiles[p][0 if which == "q" else 1]
                w = wq_sb if which == "q" else wk_sb
                ps = pool.tile([128, 512], f32, tag=TAGS[id(pool)], name="qkps")
                for cc in range(2):
                    nc.tensor.matmul(
                        ps[:, 0:width],
                        lhsT=w[cc][:, p * 128 : (p + 1) * 128],
                        rhs=x_rhs(cc, col, width),
                        start=(cc == 0),
                        stop=(cc == 1),
                    )
                nc.vector.tensor_copy(dst_t[:, col : col + width], ps[:, 0:width])

            def vt_unit(jc, pool):
                """v^T projection for key chunk jc -> vt slots (all heads)."""
                ps = pool.tile([128, HID], f32, tag=TAGS[id(pool)], name="vtps")
                for cc in range(2):
                    nc.tensor.matmul(
                        ps[:],
                        lhsT=x_rhs(cc, jc * 128, 128),
                        rhs=wv_sb[cc][:],
                        start=(cc == 0),
                        stop=(cc == 1),
                    )
                vslice = vt[
                    :, jc * H * VSLOT : (jc + 1) * H * VSLOT
                ].rearrange("p (h s) -> p h s", s=VSLOT)
                nc.vector.tensor_copy(
                    vslice[:, :, 0:DH],
                    ps[:].rearrange("p (h d) -> p h d", d=DH),
                )
                nc.vector.tensor_copy(
                    vslice[:, :, DH : DH + 1],
                    ones_sb[:].rearrange("p (h o) -> p h o", o=1),
                )

            def dots_emit(s):
                c, jc = divmod(s, 16)
                p, ic = divmod(c, 2)
                q_t, k_t = qk_tiles[p]
                d = pd.tile([128, 1024], f32, tag="dots", name=f"d{s}")
                nc.tensor.matmul(
                    d[:, 0:512],
                    lhsT=k_t[0:64, jc * 128 : (jc + 1) * 128],
                    rhs=q_t[0:64, ic * 512 : (ic + 1) * 512],
                    start=True,
                    stop=True,
                )
                nc.tensor.matmul(
                    d[:, 512:1024],
                    lhsT=k_t[64:128, jc * 128 : (jc + 1) * 128],
                    rhs=q_t[64:128, ic * 512 : (ic + 1) * 512],
                    start=True,
                    stop=True,
                )
                return d

            def exp_emit(s, d):
                e = ep.tile([128, 1024], bf16, tag="e")
                nc.scalar.activation(e[:], d[:], Exp, scale=SCALE)
                e_tiles[s] = e

            def av_emit(s):
                c, jc = divmod(s, 16)
                p, ic = divmod(c, 2)
                if jc == 0:
                    attA = pa.tile([128, 512], f32, tag="att", name=f"attA{c}")
                    attB = pa.tile([128, 512], f32, tag="att", name=f"attB{c}")
                    att_tiles[c] = (attA, attB)
                attA, attB = att_tiles[c]
                e = e_tiles[s]
                base = jc * H * VSLOT
                hA = 2 * p
                hB = 2 * p + 1
                nc.tensor.matmul(
                    attA[0 : DH + 1, :],
                    lhsT=vt[:, base + hA * VSLOT : base + (hA + 1) * VSLOT],
                    rhs=e[:, 0:512],
                    start=(jc == 0),
                    stop=(jc == 15),
                )
                nc.tensor.matmul(
                    attB[0 : DH + 1, :],
                    lhsT=vt[:, base + hB * VSLOT : base + (hB + 1) * VSLOT],
                    rhs=e[:, 512:1024],
                    start=(jc == 0),
                    stop=(jc == 15),
                )

            def norm_copy(c):
                """Free both att PSUM banks ASAP: att_s copies first (the
                staggered next-chain AVs wait on these), then head A's
                den + reciprocal."""
                attA, attB = att_tiles[c]
                att_sA = sp.tile([64, 512], f32, tag="att_s", name=f"as{c}_0")
                nc.vector.tensor_copy(att_sA[:], attA[0:64, :])
                att_sB = sp.tile([64, 512], f32, tag="att_s", name=f"as{c}_1")
                nc.vector.tensor_copy(att_sB[:], attB[0:64, :])
                denA = sp.tile([1, 512], f32, tag="den")
                nc.vector.tensor_copy(denA[:], attA[64:65, :])
                recA = sp.tile([1, 512], f32, tag="rec")
                nc.vector.reciprocal_approx_fast(recA[:], denA[:])
                norm_state[(c, 0)] = (att_sA, recA)
                norm_state[(c, 1)] = (att_sB, attB)

            def norm_den_b(c):
                """Head B's den + reciprocal (second step of the chain)."""
                att_sB, attB = norm_state[(c, 1)]
                denB = sp.tile([1, 512], f32, tag="den")
                nc.vector.tensor_copy(denB[:], attB[64:65, :])
                recB = sp.tile([1, 512], f32, tag="rec")
                nc.vector.reciprocal_approx_fast(recB[:], denB[:])
                norm_state[(c, 1)] = (att_sB, recB)

            def norm_pe(c, hh):
                """PE ones-matmul broadcast of 1/den (fp32 - PE has slack,
                and it saves a DVE cast) + DVE multiply."""
                p, ic = divmod(c, 2)
                att_s, rec = norm_state[(c, hh)]
                bc = pj.tile([64, 512], f32, tag="proj", name=f"bc{c}_{hh}")
                nc.tensor.matmul(
                    bc[:], lhsT=ones_col[:], rhs=rec[:], start=True, stop=True
                )
                nc.vector.tensor_mul(
                    attn_n[p][hh * 64 : (hh + 1) * 64, ic * 512 : (ic + 1) * 512],
                    att_s[:],
                    bc[:],
                )

            def outproj_emit(p, ic2, oc):
                ps = pj.tile([128, 512], f32, tag="proj", name=f"op{p}_{ic2}_{oc}")
                nc.tensor.matmul(
                    ps[:],
                    lhsT=wo_sb[p][:, oc * 128 : (oc + 1) * 128],
                    rhs=attn_n[p][:, ic2 * 512 : (ic2 + 1) * 512],
                    start=True,
                    stop=True,
                )
                dst = out_acc[oc][:, ic2 * 512 : (ic2 + 1) * 512]
                if p == 0:
                    nc.vector.tensor_scalar_add(dst, ps[:], bias_sb[:, oc : oc + 1])
                elif p < PAIRS - 1:
                    nc.vector.tensor_add(dst, dst, ps[:])
                else:
                    # final accumulation: column halves so each DMA (the
                    # transfer is ~2us) starts as soon as its half is added
                    for h in range(2):
                        cols = slice(h * 256, (h + 1) * 256)
                        nc.vector.tensor_add(dst[:, cols], dst[:, cols], ps[:, cols])
                        engs[(ic2 + oc + h) % 2].dma_start(
                            out_ext[
                                oc * 128 : (oc + 1) * 128,
                                ic2 * 512 + h * 256 : ic2 * 512 + (h + 1) * 256,
                            ],
                            dst[:, cols],
                        )

            TAGS = {id(pd): "dots", id(pa): "att", id(pj): "proj", id(pv): "vt"}

            # ---- per-step extra-work schedule ----
            extras = [[] for _ in range(STEPS)]

            # pair-0 remaining q/k units (pj) + vt units (own psum bank)
            extras[2].append(lambda: qk_unit(0, "k", 1024, pj))
            extras[6].append(lambda: qk_unit(0, "k", 1536, pj))
            extras[10].append(lambda: qk_unit(0, "q", 512, pj))
            extras[0].append(lambda: vt_unit(2, pv))
            extras[0].append(lambda: vt_unit(3, pv))
            for jc in range(4, 16):
                extras[jc - 3].append(lambda jc=jc: vt_unit(jc, pv))

            # qk projections for pair p: k chunk c is first consumed at
            # step 32p+4c and q_ic1 at 32p+16, so only q_ic0/k_c0/k_c1
            # must precede the pair transition; the rest spreads into the
            # pair's own first chain.
            UNIT_ORDER = [("q", 0), ("k", 0), ("k", 512),
                          ("k", 1024), ("k", 1536), ("q", 512)]
            for p in range(1, PAIRS):
                slots = [32 * p + o for o in (-15, -11, -7, 1, 5, 9)]
                for (which, col), s in zip(UNIT_ORDER, slots):
                    extras[s].append(
                        lambda p=p, which=which, col=col: qk_unit(p, which, col, pj)
                    )

            # normalize for chain c during chain c+1; out-projection for
            # chain c during chain c+2 (same parity keeps the bias-first
            # accumulation order per out_acc region; chain 6's moves late
            # into chain 7, chain 7's is the tail).
            for c in range(CHAINS - 1):
                base = 16 * (c + 1)
                extras[base + 0].append(lambda c=c: norm_copy(c))
                extras[base + 1].append(lambda c=c: norm_den_b(c))
                extras[base + 10].append(lambda c=c: norm_pe(c, 0))
                extras[base + 11].append(lambda c=c: norm_pe(c, 1))
            for c in range(CHAINS - 1):
                p, ic = divmod(c, 2)
                if c < 6:
                    s0, s1 = 16 * (c + 2) + 4, 16 * (c + 2) + 6
                else:
                    s0, s1 = 124, 126
                extras[s0].append(lambda p=p, ic=ic: outproj_emit(p, ic, 0))
                extras[s1].append(lambda p=p, ic=ic: outproj_emit(p, ic, 1))

            # ---- prologue: minimum work before dots(0).  The first k
            # unit covers only jc0's 128 columns so dots(0) starts ~4us
            # earlier; the rest of k 0:512 follows as the first extra.
            qk_unit(0, "k", 0, pd, width=128)
            qk_unit(0, "q", 0, pd)
            d_cur = dots_emit(0)
            qk_unit(0, "k", 128, pj, width=384)
            vt_unit(0, pv)
            vt_unit(1, pv)
            extras[1].insert(0, lambda: qk_unit(0, "k", 512, pj))

            # ---- main stream.  Chain-start AVs (jc==0) are deferred one
            # step so they don't head-of-line-block the PE queue while
            # waiting for the previous chain's att_s copy to free the
            # (2-buffer) attn PSUM slot. ----
            for s in range(STEPS):
                exp_emit(s, d_cur)
                if s + 1 < STEPS:
                    d_cur = dots_emit(s + 1)
                for thunk in extras[s]:
                    thunk()
                if s % 16 == 1:
                    av_emit(s - 1)
                if s % 16 != 0:
                    av_emit(s)

            # ---- tail: chain 7 normalize + final out-proj, ordered to
            # minimize the serial DVE chain (dens+recips first so the PE
            # broadcast overlaps the att_s copies) ----
            attA, attB = att_tiles[7]
            den7, rec7 = [], []
            for hh, att in ((0, attA), (1, attB)):
                den = sp.tile([1, 512], f32, tag="den")
                nc.vector.tensor_copy(den[:], att[64:65, :])
                den7.append(den)
            for hh in range(2):
                rec = sp.tile([1, 512], f32, tag="rec")
                nc.vector.reciprocal_approx_fast(rec[:], den7[hh][:])
                rec7.append(rec)
            bcA = pj.tile([64, 512], f32, tag="proj", name="bc7_0")
            nc.tensor.matmul(
                bcA[:], lhsT=ones_col[:], rhs=rec7[0][:], start=True, stop=True
            )
            att_sA = sp.tile([64, 512], f32, tag="att_s", name="as7_0")
            nc.vector.tensor_copy(att_sA[:], attA[0:64, :])
            nc.vector.tensor_mul(attn_n[3][0:64, 512:1024], att_sA[:], bcA[:])
            bcB = pj.tile([64, 512], f32, tag="proj", name="bc7_1")
            nc.tensor.matmul(
                bcB[:], lhsT=ones_col[:], rhs=rec7[1][:], start=True, stop=True
            )
            att_sB = sp.tile([64, 512], f32, tag="att_s", name="as7_1")
            nc.vector.tensor_copy(att_sB[:], attB[0:64, :])
            nc.vector.tensor_mul(attn_n[3][64:128, 512:1024], att_sB[:], bcB[:])
            outproj_emit(3, 1, 0)
            outproj_emit(3, 1, 1)

    nc.compile()
    return nc


def _shard_inputs(x, w_qkv, w_out, b_out):
    """Returns in_maps for cores 0..7; core c = (batch c//2, query-half c%2)."""
    x = np.asarray(x, dtype=np.float32)
    w_qkv = np.asarray(w_qkv, dtype=np.float32)
    w_out = np.asarray(w_out, dtype=np.float32)
    b_out = np.asarray(b_out, dtype=np.float32)

    import ml_dtypes

    bf = ml_dtypes.bfloat16
    wq_t = np.ascontiguousarray(w_qkv[0:HID].T).astype(bf)  # [256, 512]
    wk_t = np.ascontiguousarray(w_qkv[HID : 2 * HID].T).astype(bf)
    wv_t = np.ascontiguousarray(w_qkv[2 * HID : 3 * HID].T).astype(bf)
    wo_t = np.ascontiguousarray(w_out.T).astype(bf)  # [512, 256]
    bias = np.ascontiguousarray(b_out.reshape(DIM, 1))

    in_maps = []
    for c in range(8):
        b, half = divmod(c, 2)
        xb = x[b]
        halves = [xb[:, 0:NQ], xb[:, NQ:N]]
        x_perm = np.ascontiguousarray(
            np.concatenate([halves[half], halves[1 - half]], axis=1)
        ).astype(ml_dtypes.bfloat16)
        in_maps.append(
            {
                "x": x_perm,
                "wq_t": wq_t,
                "wk_t": wk_t,
                "wv_t": wv_t,
                "wo_t": wo_t,
                "bias": bias,
            }
        )
    return in_maps


def run(x, w_qkv, w_out, b_out, trace=False, tmpdir=None):
    from concourse.bass_utils import run_bass_kernel_spmd

    _register_ntff_hook()
    if "nc" not in _CACHE:
        _CACHE["nc"] = build_nc()
    nc = _CACHE["nc"]
    in_maps = _shard_inputs(x, w_qkv, w_out, b_out)
    kw = {}
    if trace:
        kw.update(trace=True, tmpdir=tmpdir)
    res = run_bass_kernel_spmd(nc, in_maps, core_ids=list(range(8)), **kw)
    out = np.empty((4, DIM, N), dtype=np.float32)
    for c in range(8):
        b, half = divmod(c, 2)
        out[b][:, half * NQ : (half + 1) * NQ] = res.results[c]["out"]
    return out, res


def kernel(**inputs):
    out, _ = run(
        inputs["x"], inputs["w_qkv"], inputs["w_out"], inputs["b_out"]
    )
    return out


# revision 64
# speedup vs baseline: 1.0471x; 1.0471x over previous
"""Distributed (8-core) Trainium2 Bass kernel for nn_Attention.

Reference computation (per batch b of 4, x: [4, 256, 2048]):
  qkv = w_qkv @ x[b]            -> q,k,v each [8 heads, 64, 2048]
  dots = (q^T k) * 64**-0.5     -> [8, 2048, 2048]
  attn = softmax(dots, -1)
  av   = v @ attn^T             -> [8, 64, 2048]
  out  = w_out @ av + b_out     -> [256, 2048]

Sharding: 8 shards = (batch b in 0..3) x (query-half in 0..1). Each core
gets the full x[b] (columns permuted so its own 1024 query positions come
first), computes full k/v (duplicated with its half-partner), q only for
its 1024 queries, its half of the attention, and its half of the final
projection. Host concatenates.

The kernel is ACT-bound: 128 exp ACTIVATEs of [128,1024] at ~1.11us each
(softmax exp over 16.8M scores/core) = 142us of ScalarE work. Everything
else is scheduled to keep that stream gapless:
  - flat 128-step stream over (pair, ic, jc); per step s the emission
    order is exp(s), dots(s+1), extras, AV(s) so the next exp's input
    (dots) never queues behind the e(s)-dependent AV matmuls in the
    in-order PE queue.
  - q/k projections, v^T projections, out-projections and normalize
    broadcasts are interleaved 1-2 matmuls per step into PE slack
    instead of bursts.
  - softmax denominators ride as a ones-column in the AV stationary
    operand; normalization is copy-out (frees the PSUM bank), recip
    (DVE), bf16 cast, ones-matmul partition broadcast (PE), multiply
    (DVE). No GPSIMD on the critical path.
"""

import sys

sys.path.insert(0, "/opt/trn_rl_repo")
sys.path.insert(0, "/root/.axon_site")

import numpy as np

DIM = 256
N = 2048
NQ = 1024
H = 8
DH = 64
HID = 512
PAIRS = 4
CHAINS = 8  # (pair, ic)
STEPS = CHAINS * 16
SCALE = DH ** -0.5

_CACHE = {}


def _register_ntff_hook():
    """The agent image's antenv lacks axon_hooks; synthesize it so
    run_bass_kernel_spmd(trace=True) can profile. Harmless if unused."""
    import types

    if "antenv.axon_hooks" in sys.modules:
        return
    try:
        import antenv
        from trn_agent_boot.trn_boot import _ntff_profile_via_ctypes

        mod = types.ModuleType("antenv.axon_hooks")
        _hook = [None]
        mod.set_axon_ntff_profile_hook = lambda h: _hook.__setitem__(0, h)
        mod.get_axon_ntff_profile_hook = lambda: _hook[0]
        sys.modules["antenv.axon_hooks"] = mod
        antenv.axon_hooks = mod
        mod.set_axon_ntff_profile_hook(
            _ntff_profile_via_ctypes("/opt/axon/libaxon_pjrt.so")
        )
    except Exception:
        pass


def build_nc():
    import concourse.mybir as mybir
    import concourse.tile as tile
    from concourse import bacc

    f32 = mybir.dt.float32
    bf16 = mybir.dt.bfloat16
    Exp = mybir.ActivationFunctionType.Exp

    nc = bacc.Bacc("TRN2", target_bir_lowering=False, debug=False)

    x_ext = nc.dram_tensor("x", [DIM, N], bf16, kind="ExternalInput")
    wq_ext = nc.dram_tensor("wq_t", [DIM, HID], bf16, kind="ExternalInput")
    wk_ext = nc.dram_tensor("wk_t", [DIM, HID], bf16, kind="ExternalInput")
    wv_ext = nc.dram_tensor("wv_t", [DIM, HID], bf16, kind="ExternalInput")
    wo_ext = nc.dram_tensor("wo_t", [HID, DIM], bf16, kind="ExternalInput")
    b_ext = nc.dram_tensor("bias", [DIM, 1], f32, kind="ExternalInput")
    out_ext = nc.dram_tensor("out", [DIM, NQ], f32, kind="ExternalOutput")

    VSLOT = DH + 1  # 64 v columns + 1 ones column per head

    with tile.TileContext(nc) as tc:
        with (
            tc.tile_pool(name="persist", bufs=1) as pp,
            tc.tile_pool(name="qk", bufs=2) as qk,
            tc.tile_pool(name="epool", bufs=20) as ep,
            tc.tile_pool(name="small", bufs=4) as sp,
            tc.tile_pool(name="pdots", bufs=2, space="PSUM") as pd,
            tc.tile_pool(name="pattn", bufs=2, space="PSUM") as pa,
            tc.tile_pool(name="pproj", bufs=1, space="PSUM") as pj,
            tc.tile_pool(name="pvt", bufs=1, space="PSUM") as pv,
        ):
            # ---- warm the ACT exp table early (one tiny op) ----
            dummy = sp.tile([1, 1], f32, tag="dummy")
            nc.vector.memset(dummy[:], 0.0)
            dummy2 = sp.tile([1, 1], f32, tag="dummy2")
            nc.scalar.activation(dummy2[:], dummy[:], Exp)

            # ---- input DMAs: wq + x head first so the q projection can
            # start ASAP; wk/wv next (k proj, v proj); bulk x after.
            engs = [nc.sync, nc.gpsimd]
            wq_sb = [pp.tile([128, HID], bf16, tag=f"wq{c}", name=f"wq{c}") for c in range(2)]
            wk_sb = [pp.tile([128, HID], bf16, tag=f"wk{c}", name=f"wk{c}") for c in range(2)]
            wv_sb = [pp.tile([128, HID], bf16, tag=f"wv{c}", name=f"wv{c}") for c in range(2)]
            # x split into 3 column-group tiles per row chunk: dependency
            # tracking is tile-granular, so a single [128,2048] tile would
            # make every consumer wait for ALL of x's DMAs.
            XW = [512, 512, 1024]
            XO = [0, 512, 1024]
            xg = [
                [
                    pp.tile([128, XW[g]], bf16, tag=f"x{c}_{g}", name=f"x{c}_{g}")
                    for g in range(3)
                ]
                for c in range(2)
            ]

            def x_rhs(cc, col, width):
                g = 0 if col < 512 else (1 if col < 1024 else 2)
                off = col - XO[g]
                return xg[cc][g][:, off : off + width]

            # 3 DGE queues; the q/k projection inputs (wq, xA, wk) are the
            # startup critical path, one ~256KB set per queue.
            for c in range(2):
                engs[c].dma_start(wq_sb[c][:], wq_ext[c * 128 : (c + 1) * 128, :])
            for c in range(2):
                engs[c].dma_start(
                    xg[c][0][:], x_ext[c * 128 : (c + 1) * 128, 0:512]
                )
            for c in range(2):
                engs[c].dma_start(wk_sb[c][:], wk_ext[c * 128 : (c + 1) * 128, :])
            for c in range(2):
                engs[c].dma_start(wv_sb[c][:], wv_ext[c * 128 : (c + 1) * 128, :])
            for c in range(2):
                engs[c].dma_start(
                    xg[c][1][:], x_ext[c * 128 : (c + 1) * 128, 512:1024]
                )
            for c in range(2):
                engs[c].dma_start(
                    xg[c][2][:], x_ext[c * 128 : (c + 1) * 128, 1024:2048]
                )
            wo_sb = []
            for cc in range(4):
                t = pp.tile([128, DIM], bf16, tag=f"wo{cc}", name=f"wo{cc}")
                engs[cc % 2].dma_start(t[:], wo_ext[cc * 128 : (cc + 1) * 128, :])
                wo_sb.append(t)
            bias_sb = pp.tile([128, 2], f32, tag="bias")
            for oc in range(2):
                nc.sync.dma_start(
                    bias_sb[:, oc : oc + 1], b_ext[oc * 128 : (oc + 1) * 128, :]
                )

            # ---- persistent SBUF state ----
            ones_col = pp.tile([1, DH], bf16, tag="ones_col")
            nc.vector.memset(ones_col[:], 1.0)
            ones_sb = pp.tile([128, H], f32, tag="ones")
            nc.vector.memset(ones_sb[:], 1.0)
            vt = pp.tile([128, 16 * H * VSLOT], bf16, tag="vt")

            attn_n = [
                pp.tile([128, NQ], bf16, tag=f"attn_n{p}", name=f"attn_n{p}")
                for p in range(PAIRS)
            ]
            out_acc = [
                pp.tile([128, NQ], f32, tag=f"oacc{oc}", name=f"oacc{oc}")
                for oc in range(2)
            ]

            # chain bookkeeping: chain c = (pair c//2, ic c%2)
            qk_tiles = [None] * PAIRS
            att_tiles = [None] * CHAINS
            e_tiles = [None] * STEPS
            norm_state = {}

            # ---- helper unit emitters ----
            def qk_unit(p, which, col, pool, width=512):
                """One q/k projection unit for pair p: [128,width] + cast."""
                if qk_tiles[p] is None:
                    qk_tiles[p] = (
                        qk.tile([128, NQ], bf16, tag="q", name=f"q{p}"),
                        qk.tile([128, N], bf16, tag="k", name=f"k{p}"),
                    )
                dst_t = qk_tiles[p][0 if which == "q" else 1]
                w = wq_sb if which == "q" else wk_sb
                ps = pool.tile([128, 512], f32, tag=TAGS[id(pool)], name="qkps")
                for cc in range(2):
                    nc.tensor.matmul(
                        ps[:, 0:width],
                        lhsT=w[cc][:, p * 128 : (p + 1) * 128],
                        rhs=x_rhs(cc, col, width),
                        start=(cc == 0),
                        stop=(cc == 1),
                    )
                nc.vector.tensor_copy(dst_t[:, col : col + width], ps[:, 0:width])

            def vt_unit(jc, pool):
                """v^T projection for key chunk jc -> vt slots (all heads)."""
                ps = pool.tile([128, HID], f32, tag=TAGS[id(pool)], name="vtps")
                for cc in range(2):
                    nc.tensor.matmul(
                        ps[:],
                        lhsT=x_rhs(cc, jc * 128, 128),
                        rhs=wv_sb[cc][:],
                        start=(cc == 0),
                        stop=(cc == 1),
                    )
                vslice = vt[
                    :, jc * H * VSLOT : (jc + 1) * H * VSLOT
                ].rearrange("p (h s) -> p h s", s=VSLOT)
                nc.vector.tensor_copy(
                    vslice[:, :, 0:DH],
                    ps[:].rearrange("p (h d) -> p h d", d=DH),
                )
                nc.vector.tensor_copy(
                    vslice[:, :, DH : DH + 1],
                    ones_sb[:].rearrange("p (h o) -> p h o", o=1),
                )

            def dots_emit(s):
                c, jc = divmod(s, 16)
                p, ic = divmod(c, 2)
                q_t, k_t = qk_tiles[p]
                d = pd.tile([128, 1024], f32, tag="dots", name=f"d{s}")
                nc.tensor.matmul(
                    d[:, 0:512],
                    lhsT=k_t[0:64, jc * 128 : (jc + 1) * 128],
                    rhs=q_t[0:64, ic * 512 : (ic + 1) * 512],
                    start=True,
                    stop=True,
                )
                nc.tensor.matmul(
                    d[:, 512:1024],
                    lhsT=k_t[64:128, jc * 128 : (jc + 1) * 128],
                    rhs=q_t[64:128, ic * 512 : (ic + 1) * 512],
                    start=True,
                    stop=True,
                )
                return d

            def exp_emit(s, d):
                e = ep.tile([128, 1024], bf16, tag="e")
                nc.scalar.activation(e[:], d[:], Exp, scale=SCALE)
                e_tiles[s] = e

            def av_emit(s):
                c, jc = divmod(s, 16)
                p, ic = divmod(c, 2)
                if jc == 0:
                    attA = pa.tile([128, 512], f32, tag="att", name=f"attA{c}")
                    attB = pa.tile([128, 512], f32, tag="att", name=f"attB{c}")
                    att_tiles[c] = (attA, attB)
                attA, attB = att_tiles[c]
                e = e_tiles[s]
                base = jc * H * VSLOT
                hA = 2 * p
                hB = 2 * p + 1
                nc.tensor.matmul(
                    attA[0 : DH + 1, :],
                    lhsT=vt[:, base + hA * VSLOT : base + (hA + 1) * VSLOT],
                    rhs=e[:, 0:512],
                    start=(jc == 0),
                    stop=(jc == 15),
                )
                nc.tensor.matmul(
                    attB[0 : DH + 1, :],
                    lhsT=vt[:, base + hB * VSLOT : base + (hB + 1) * VSLOT],
                    rhs=e[:, 512:1024],
                    start=(jc == 0),
                    stop=(jc == 15),
                )

            def norm_copy(c):
                """Free both att PSUM banks ASAP: att_s copies first (the
                staggered next-chain AVs wait on these), then head A's
                den + reciprocal."""
                attA, attB = att_tiles[c]
                att_sA = sp.tile([64, 512], f32, tag="att_s", name=f"as{c}_0")
                nc.vector.tensor_copy(att_sA[:], attA[0:64, :])
                att_sB = sp.tile([64, 512], f32, tag="att_s", name=f"as{c}_1")
                nc.vector.tensor_copy(att_sB[:], attB[0:64, :])
                denA = sp.tile([1, 512], f32, tag="den")
                nc.vector.tensor_copy(denA[:], attA[64:65, :])
                recA = sp.tile([1, 512], f32, tag="rec")
                nc.vector.reciprocal_approx_fast(recA[:], denA[:])
                norm_state[(c, 0)] = (att_sA, recA)
                norm_state[(c, 1)] = (att_sB, attB)

            def norm_den_b(c):
                """Head B's den + reciprocal (second step of the chain)."""
                att_sB, attB = norm_state[(c, 1)]
                denB = sp.tile([1, 512], f32, tag="den")
                nc.vector.tensor_copy(denB[:], attB[64:65, :])
                recB = sp.tile([1, 512], f32, tag="rec")
                nc.vector.reciprocal_approx_fast(recB[:], denB[:])
                norm_state[(c, 1)] = (att_sB, recB)

            def norm_pe(c, hh):
                """PE ones-matmul broadcast of 1/den + DVE multiply."""
                p, ic = divmod(c, 2)
                att_s, rec = norm_state[(c, hh)]
                recb = sp.tile([1, 512], bf16, tag="recb")
                nc.vector.tensor_copy(recb[:], rec[:])
                bc = pj.tile([64, 512], f32, tag="proj", name=f"bc{c}_{hh}")
                nc.tensor.matmul(
                    bc[:], lhsT=ones_col[:], rhs=recb[:], start=True, stop=True
                )
                nc.vector.tensor_mul(
                    attn_n[p][hh * 64 : (hh + 1) * 64, ic * 512 : (ic + 1) * 512],
                    att_s[:],
                    bc[:],
                )

            def outproj_emit(p, ic2, oc):
                ps = pj.tile([128, 512], f32, tag="proj", name=f"op{p}_{ic2}_{oc}")
                nc.tensor.matmul(
                    ps[:],
                    lhsT=wo_sb[p][:, oc * 128 : (oc + 1) * 128],
                    rhs=attn_n[p][:, ic2 * 512 : (ic2 + 1) * 512],
                    start=True,
                    stop=True,
                )
                dst = out_acc[oc][:, ic2 * 512 : (ic2 + 1) * 512]
                if p == 0:
                    nc.vector.tensor_scalar_add(dst, ps[:], bias_sb[:, oc : oc + 1])
                elif p < PAIRS - 1:
                    nc.vector.tensor_add(dst, dst, ps[:])
                else:
                    # final accumulation: column halves so each DMA (the
                    # transfer is ~2us) starts as soon as its half is added
                    for h in range(2):
                        cols = slice(h * 256, (h + 1) * 256)
                        nc.vector.tensor_add(dst[:, cols], dst[:, cols], ps[:, cols])
                        engs[(ic2 + oc + h) % 2].dma_start(
                            out_ext[
                                oc * 128 : (oc + 1) * 128,
                                ic2 * 512 + h * 256 : ic2 * 512 + (h + 1) * 256,
                            ],
                            dst[:, cols],
                        )

            TAGS = {id(pd): "dots", id(pa): "att", id(pj): "proj", id(pv): "vt"}

            # ---- per-step extra-work schedule ----
            extras = [[] for _ in range(STEPS)]

            # pair-0 remaining q/k units (pj) + vt units (own psum bank)
            extras[2].append(lambda: qk_unit(0, "k", 1024, pj))
            extras[6].append(lambda: qk_unit(0, "k", 1536, pj))
            extras[10].append(lambda: qk_unit(0, "q", 512, pj))
            extras[0].append(lambda: vt_unit(2, pv))
            extras[0].append(lambda: vt_unit(3, pv))
            for jc in range(4, 16):
                extras[jc - 3].append(lambda jc=jc: vt_unit(jc, pv))

            # qk projections for pair p: k chunk c is first consumed at
            # step 32p+4c and q_ic1 at 32p+16, so only q_ic0/k_c0/k_c1
            # must precede the pair transition; the rest spreads into the
            # pair's own first chain.
            UNIT_ORDER = [("q", 0), ("k", 0), ("k", 512),
                          ("k", 1024), ("k", 1536), ("q", 512)]
            for p in range(1, PAIRS):
                slots = [32 * p + o for o in (-15, -11, -7, 1, 5, 9)]
                for (which, col), s in zip(UNIT_ORDER, slots):
                    extras[s].append(
                        lambda p=p, which=which, col=col: qk_unit(p, which, col, pj)
                    )

            # normalize for chain c during chain c+1; out-projection for
            # chain c during chain c+2 (same parity keeps the bias-first
            # accumulation order per out_acc region; chain 6's moves late
            # into chain 7, chain 7's is the tail).
            for c in range(CHAINS - 1):
                base = 16 * (c + 1)
                extras[base + 0].append(lambda c=c: norm_copy(c))
                extras[base + 1].append(lambda c=c: norm_den_b(c))
                extras[base + 10].append(lambda c=c: norm_pe(c, 0))
                extras[base + 11].append(lambda c=c: norm_pe(c, 1))
            for c in range(CHAINS - 1):
                p, ic = divmod(c, 2)
                if c < 6:
                    s0, s1 = 16 * (c + 2) + 4, 16 * (c + 2) + 6
                else:
                    s0, s1 = 124, 126
                extras[s0].append(lambda p=p, ic=ic: outproj_emit(p, ic, 0))
                extras[s1].append(lambda p=p, ic=ic: outproj_emit(p, ic, 1))

            # ---- prologue: minimum work before dots(0).  The first k
            # unit covers only jc0's 128 columns so dots(0) starts ~4us
            # earlier; the rest of k 0:512 follows as the first extra.
            qk_unit(0, "k", 0, pd, width=128)
            qk_unit(0, "q", 0, pd)
            d_cur = dots_emit(0)
            qk_unit(0, "k", 128, pj, width=384)
            vt_unit(0, pv)
            vt_unit(1, pv)
            extras[1].insert(0, lambda: qk_unit(0, "k", 512, pj))

            # ---- main stream.  Chain-start AVs (jc==0) are deferred one
            # step so they don't head-of-line-block the PE queue while
            # waiting for the previous chain's att_s copy to free the
            # (2-buffer) attn PSUM slot. ----
            for s in range(STEPS):
                exp_emit(s, d_cur)
                if s + 1 < STEPS:
                    d_cur = dots_emit(s + 1)
                for thunk in extras[s]:
                    thunk()
                if s % 16 == 1:
                    av_emit(s - 1)
                if s % 16 != 0:
                    av_emit(s)

            # ---- tail: chain 7 normalize + final out-proj, ordered to
            # minimize the serial DVE chain (dens+recips first so the PE
            # broadcast overlaps the att_s copies) ----
            attA, attB = att_tiles[7]
            den7, rec7 = [], []
            for hh, att in ((0, attA), (1, attB)):
                den = sp.tile([1, 512], f32, tag="den")
                nc.vector.tensor_copy(den[:], att[64:65, :])
                den7.append(den)
            for hh in range(2):
                rec = sp.tile([1, 512], f32, tag="rec")
                nc.vector.reciprocal_approx_fast(rec[:], den7[hh][:])
                rec7.append(rec)
            recbA = sp.tile([1, 512], bf16, tag="recb")
            nc.vector.tensor_copy(recbA[:], rec7[0][:])
            bcA = pj.tile([64, 512], f32, tag="proj", name="bc7_0")
            nc.tensor.matmul(
                bcA[:], lhsT=ones_col[:], rhs=recbA[:], start=True, stop=True
            )
            recbB = sp.tile([1, 512], bf16, tag="recb")
            nc.vector.tensor_copy(recbB[:], rec7[1][:])
            att_sA = sp.tile([64, 512], f32, tag="att_s", name="as7_0")
            nc.vector.tensor_copy(att_sA[:], attA[0:64, :])
            nc.vector.tensor_mul(attn_n[3][0:64, 512:1024], att_sA[:], bcA[:])
            bcB = pj.tile([64, 512], f32, tag="proj", name="bc7_1")
            nc.tensor.matmul(
                bcB[:], lhsT=ones_col[:], rhs=recbB[:], start=True, stop=True
            )
            att_sB = sp.tile([64, 512], f32, tag="att_s", name="as7_1")
            nc.vector.tensor_copy(att_sB[:], attB[0:64, :])
            nc.vector.tensor_mul(attn_n[3][64:128, 512:1024], att_sB[:], bcB[:])
            outproj_emit(3, 1, 0)
            outproj_emit(3, 1, 1)

    nc.compile()
    return nc


def _shard_inputs(x, w_qkv, w_out, b_out):
    """Returns in_maps for cores 0..7; core c = (batch c//2, query-half c%2)."""
    x = np.asarray(x, dtype=np.float32)
    w_qkv = np.asarray(w_qkv, dtype=np.float32)
    w_out = np.asarray(w_out, dtype=np.float32)
    b_out = np.asarray(b_out, dtype=np.float32)

    import ml_dtypes

    bf = ml_dtypes.bfloat16
    wq_t = np.ascontiguousarray(w_qkv[0:HID].T).astype(bf)  # [256, 512]
    wk_t = np.ascontiguousarray(w_qkv[HID : 2 * HID].T).astype(bf)
    wv_t = np.ascontiguousarray(w_qkv[2 * HID : 3 * HID].T).astype(bf)
    wo_t = np.ascontiguousarray(w_out.T).astype(bf)  # [512, 256]
    bias = np.ascontiguousarray(b_out.reshape(DIM, 1))

    in_maps = []
    for c in range(8):
        b, half = divmod(c, 2)
        xb = x[b]
        halves = [xb[:, 0:NQ], xb[:, NQ:N]]
        x_perm = np.ascontiguousarray(
            np.concatenate([halves[half], halves[1 - half]], axis=1)
        ).astype(ml_dtypes.bfloat16)
        in_maps.append(
            {
                "x": x_perm,
                "wq_t": wq_t,
                "wk_t": wk_t,
                "wv_t": wv_t,
                "wo_t": wo_t,
                "bias": bias,
            }
        )
    return in_maps


def run(x, w_qkv, w_out, b_out, trace=False, tmpdir=None):
    from concourse.bass_utils import run_bass_kernel_spmd

    _register_ntff_hook()
    if "nc" not in _CACHE:
        _CACHE["nc"] = build_nc()
    nc = _CACHE["nc"]
    in_maps = _shard_inputs(x, w_qkv, w_out, b_out)
    kw = {}
    if trace:
        kw.update(trace=True, tmpdir=tmpdir)
    res = run_bass_kernel_spmd(nc, in_maps, core_ids=list(range(8)), **kw)
    out = np.empty((4, DIM, N), dtype=np.float32)
    for c in range(8):
        b, half = divmod(c, 2)
        out[b][:, half * NQ : (half + 1) * NQ] = res.results[c]["out"]
    return out, res


def kernel(**inputs):
    out, _ = run(
        inputs["x"], inputs["w_qkv"], inputs["w_out"], inputs["b_out"]
    )
    return out
